# revision 1
# baseline (speedup 1.0000x reference)
"""GQA kernel for Trainium2, 8 NeuronCores (DP over batch x TP over heads).

Problem (hardcoded): B=4, S=1024, EMBED=2048, HEADS=32, GROUPS=8,
GROUP_HEADS=4, HEAD_DIM=64.

Sharding: core c handles batch b = c//2 and TP half m = c%2
(16 heads = 4 groups per core). All tensors are pre-transposed on the
host so the device only ever sees contract-dim-major operands:

  qT/kT/vT  [2048, 1024]   (embed-major tokens for one batch)
  wqT       [2048, 1024]   (Wq rows m*1024:(m+1)*1024, transposed, pre-scaled 1/8)
  wkT/wvT   [2048, 256]    (Wk/Wv rows m*256:(m+1)*256, transposed)
  wfcT      [1024, 2048]   (Wfc columns m*1024:(m+1)*1024, transposed)
  y         [1024, 2048]   partial output; host computes y[2b] + y[2b+1] + bfc.

Device pipeline per core (all matmuls fp32r):
  1. khT [256,1024] and vh [1024,256] projections; khT duplicated into
     per-group [128,1024] tiles (both 64-partition halves hold the same
     group) so score matmuls can run at either array quadrant; vh gets
     a ones column (AV matmul then emits softmax denominators for free).
  2. qhT [1024,1024] projection (head-dim-major).
  3. Per head: scores = khT_g.T @ qhT_h -> exp (ACT, no max subtraction:
     |score| <= ~6 by construction) -> AV accumulation (vh as stationary)
     -> normalize by denominator broadcast (DMA partition-replicate +
     DVE reciprocal/mul) into OT [1024,1024].
  4. y = OT.T @ wfcT accumulated over 8 i-chunks.
"""

import numpy as np

import concourse.bass as bass
import concourse.tile as tile
from concourse import bacc, mybir
from concourse.bass_utils import run_bass_kernel_spmd

F32 = mybir.dt.float32
F32R = mybir.dt.float32r
AF = mybir.ActivationFunctionType

B, S, E = 4, 1024, 2048
HEADS_L = 16          # heads per core
GROUPS_L = 4          # groups per core
D = 64                # head dim
P = 128
NE = E // P           # 16 e-chunks
NT = S // P           # 8 token chunks
HO = HEADS_L * D      # 1024 local head-dims
GO = GROUPS_L * D     # 256 local group-dims

_CACHE = {}


def _build():
    nc = bacc.Bacc("TRN2")
    qT = nc.declare_dram_parameter("qT", [E, S], F32R, isOutput=False)
    kT = nc.declare_dram_parameter("kT", [E, S], F32R, isOutput=False)
    vT = nc.declare_dram_parameter("vT", [E, S], F32R, isOutput=False)
    wqT = nc.declare_dram_parameter("wqT", [E, HO], F32R, isOutput=False)
    wkT = nc.declare_dram_parameter("wkT", [E, GO], F32R, isOutput=False)
    wvT = nc.declare_dram_parameter("wvT", [E, GO], F32R, isOutput=False)
    wfcT = nc.declare_dram_parameter("wfcT", [HO, E], F32R, isOutput=False)
    y = nc.declare_dram_parameter("y", [S, E], F32, isOutput=True)
    dbg = {}
    if _CACHE.get("debug"):
        for nm, shp in [("dqh", [P, S]), ("dkh", [P, S]), ("dvh", [P, GROUPS_L * (D + 1)]),
                        ("dexp", [P, S]), ("drecip", [P, S]), ("dot", [P, S])]:
            dbg[nm] = nc.declare_dram_parameter(nm, shp, F32, isOutput=True)

    with tile.TileContext(nc) as tc:
        _body(nc, tc, qT, kT, vT, wqT, wkT, wvT, wfcT, y, dbg)
    nc.finalize()
    return nc


def _body(nc, tc, qT, kT, vT, wqT, wkT, wvT, wfcT, y, dbg=None):
    dbg = dbg or {}
    from contextlib import ExitStack
    with ExitStack() as ctx:
        # persistent pools (whole kernel lifetime)
        p_kh = ctx.enter_context(tc.tile_pool(name="kh", bufs=GROUPS_L))
        p_vh = ctx.enter_context(tc.tile_pool(name="vh", bufs=NT))
        p_qh = ctx.enter_context(tc.tile_pool(name="qh", bufs=NT))
        p_ot = ctx.enter_context(tc.tile_pool(name="ot", bufs=NT))
        p_wfc = ctx.enter_context(tc.tile_pool(name="wfc", bufs=8))
        p_y = ctx.enter_context(tc.tile_pool(name="y", bufs=2))
        ps = ctx.enter_context(tc.tile_pool(name="ps", bufs=8, space="PSUM"))

        kh_dup = [p_kh.tile([P, S], F32R, tag="kh", name=f"khdup_{g}") for g in range(GROUPS_L)]
        vh_aug = [p_vh.tile([P, GROUPS_L, D + 1], F32R, tag="vh", name=f"vhaug_{t}")
                  for t in range(NT)]
        qh_t = [p_qh.tile([P, S], F32R, tag="qh", name=f"qh_{t}") for t in range(NT)]
        ot_t = [p_ot.tile([P, S], F32R, tag="ot", name=f"ot_{t}") for t in range(NT)]

        with tc.tile_pool(name="wk", bufs=NE) as p_wk, \
             tc.tile_pool(name="wv", bufs=NE) as p_wv, \
             tc.tile_pool(name="kv", bufs=4) as p_kv, \
             tc.tile_pool(name="wq", bufs=4) as p_wq:
            # ---- stage 1: K/V projections ---------------------------
            wk_t = []
            wv_t = []
            kh_ps = [[ps.tile([P, 512], F32, tag="ps", name=f"khps_{a}_{b}") for b in range(2)]
                     for a in range(2)]
            for e in range(NE):
                wkt = p_wk.tile([P, GO], F32R, tag="wk")
                nc.sync.dma_start(out=wkt, in_=wkT[e * P:(e + 1) * P, :])
                wk_t.append(wkt)
                kte = p_kv.tile([P, S], F32R, tag="kv", name=f"kte_{e}")
                nc.sync.dma_start(out=kte, in_=kT[e * P:(e + 1) * P, :])
                wvt = p_wv.tile([P, GO], F32R, tag="wv")
                nc.sync.dma_start(out=wvt, in_=wvT[e * P:(e + 1) * P, :])
                wv_t.append(wvt)
                for o2 in range(2):
                    for t2 in range(2):
                        nc.tensor.matmul(
                            kh_ps[o2][t2][:, :],
                            wk_t[e][:, o2 * P:(o2 + 1) * P],
                            kte[:, t2 * 512:(t2 + 1) * 512],
                            start=(e == 0), stop=(e == NE - 1),
                        )
            for o2 in range(2):
                for t2 in range(2):
                    sl = slice(t2 * 512, (t2 + 1) * 512)
                    nc.vector.tensor_copy(kh_dup[2 * o2][0:D, sl],
                                          kh_ps[o2][t2][0:D, :])
                    nc.vector.tensor_copy(kh_dup[2 * o2 + 1][D:P, sl],
                                          kh_ps[o2][t2][D:P, :])
            for g in range(GROUPS_L):
                if g % 2 == 0:
                    nc.gpsimd.dma_start(out=kh_dup[g][D:P, :], in_=kh_dup[g][0:D, :])
                else:
                    nc.gpsimd.dma_start(out=kh_dup[g][0:D, :], in_=kh_dup[g][D:P, :])

            vh_ps = [ps.tile([P, GO], F32, tag="ps", name=f"vhps_{t}") for t in range(NT)]
            for e in range(NE):
                vte = p_kv.tile([P, S], F32R, tag="kv", name=f"vte_{e}")
                nc.sync.dma_start(out=vte, in_=vT[e * P:(e + 1) * P, :])
                for t in range(NT):
                    nc.tensor.matmul(
                        vh_ps[t][:, :],
                        vte[:, t * P:(t + 1) * P],
                        wv_t[e][:, :],
                        start=(e == 0), stop=(e == NE - 1),
                    )
            for t in range(NT):
                for g in range(GROUPS_L):
                    nc.vector.tensor_copy(vh_aug[t][:, g, 0:D],
                                          vh_ps[t][:, g * D:(g + 1) * D])
                ones = nc.const_aps.tensor(1.0, (P, 1), F32)
                for g in range(GROUPS_L):
                    nc.vector.tensor_copy(vh_aug[t][:, g, D:D + 1], ones)

            # ---- stage 2: Q projection ------------------------------
            for rnd in range(2):
                wq_r = []
                for e in range(NE):
                    wqe = p_wq.tile([P, 512], F32R, tag="wq", name=f"wq_{rnd}_{e}")
                    nc.sync.dma_start(
                        out=wqe,
                        in_=wqT[e * P:(e + 1) * P, rnd * 512:(rnd + 1) * 512])
                    wq_r.append(wqe)
                qps = [[ps.tile([P, 512], F32, tag="ps", name=f"qps_{a}_{b}") for b in range(2)]
                       for a in range(4)]
                for e in range(NE):
                    qte = p_kv.tile([P, S], F32R, tag="kv", name=f"qte_{rnd}_{e}")
                    nc.sync.dma_start(out=qte, in_=qT[e * P:(e + 1) * P, :])
                    for o in range(4):
                        for t2 in range(2):
                            nc.tensor.matmul(
                                qps[o][t2][:, :],
                                wq_r[e][:, o * P:(o + 1) * P],
                                qte[:, t2 * 512:(t2 + 1) * 512],
                                start=(e == 0), stop=(e == NE - 1),
                            )
                for o in range(4):
                    for t2 in range(2):
                        nc.scalar.activation(
                            qh_t[rnd * 4 + o][:, t2 * 512:(t2 + 1) * 512],
                            qps[o][t2][:, :], AF.Copy)

        if dbg:
            nc.sync.dma_start(out=dbg["dqh"][:, :], in_=qh_t[0][:, :].bitcast(F32))
            nc.sync.dma_start(out=dbg["dkh"][:, :], in_=kh_dup[0][:, :].bitcast(F32))
            nc.sync.dma_start(out=dbg["dvh"][:, :], in_=vh_aug[0].rearrange("p g d -> p (g d)").bitcast(F32))

        # ---- stage 3: attention per head ----------------------------
        with tc.tile_pool(name="exp", bufs=10) as p_exp, \
             tc.tile_pool(name="sm", bufs=3) as p_sm:
            for h in range(HEADS_L):
                g = h // 4
                qtile = qh_t[h // 2]
                qb = (h % 2) * D  # partition base inside qh tile

                exp_t = [p_exp.tile([P, S], F32R, tag="exp", name=f"exp_{h}_{kc}") for kc in range(NT)]
                for kc in range(NT):
                    for q2 in range(2):
                        sps = ps.tile([P, 512], F32, tag="ps", name=f"sps_{h}_{kc}_{q2}")
                        nc.tensor.matmul(
                            sps[:, :],
                            kh_dup[g][qb:qb + D, kc * P:(kc + 1) * P],
                            qtile[qb:qb + D, q2 * 512:(q2 + 1) * 512],
                            start=True, stop=True,
                        )
                        nc.scalar.activation(
                            exp_t[kc][:, q2 * 512:(q2 + 1) * 512], sps[:, :],
                            AF.Exp)

                den = p_sm.tile([P, S], F32, tag="den", name=f"den_{h}")
                av_ps = []
                for q2 in range(2):
                    ops = ps.tile([P, 512], F32, tag="ps", name=f"avps_{h}_{q2}")
                    for kc in range(NT):
                        nc.tensor.matmul(
                            ops[0:D + 1, :],
                            vh_aug[kc][:, g, :],
                            exp_t[kc][:, q2 * 512:(q2 + 1) * 512],
                            start=(kc == 0), stop=(kc == NT - 1),
                        )
                    nc.vector.tensor_copy(den[D:D + 1, q2 * 512:(q2 + 1) * 512],
                                          ops[D:D + 1, :])
                    av_ps.append(ops)
                recip = p_sm.tile([P, S], F32, tag="recip", name=f"recip_{h}")
                nc.gpsimd.dma_start(out=den[0:1, :], in_=den[D:D + 1, :])
                nc.gpsimd.partition_broadcast(recip[0:D, :], den[0:1, :])
                nc.vector.reciprocal(recip[0:D, :], recip[0:D, :])
                if dbg and h == 0:
                    nc.sync.dma_start(out=dbg["dexp"][:, :], in_=exp_t[0][:, :].bitcast(F32))
                    nc.sync.dma_start(out=dbg["drecip"][:, :], in_=recip[:, :])
                if h % 2 == 0:
                    for q2 in range(2):
                        sl = slice(q2 * 512, (q2 + 1) * 512)
                        nc.vector.tensor_mul(ot_t[h // 2][0:D, sl],
                                             av_ps[q2][0:D, :], recip[0:D, sl])
                else:
                    tmp = p_sm.tile([P, S], F32R, tag="tmp", name=f"tmp_{h}")
                    for q2 in range(2):
                        sl = slice(q2 * 512, (q2 + 1) * 512)
                        nc.vector.tensor_mul(tmp[0:D, sl],
                                             av_ps[q2][0:D, :], recip[0:D, sl])
                    nc.gpsimd.dma_start(out=ot_t[h // 2][D:P, :], in_=tmp[0:D, :])

            if dbg:
                nc.sync.dma_start(out=dbg["dot"][:, :], in_=ot_t[0][:, :].bitcast(F32))

        # ---- stage 4: output projection (four out-quarter rounds) ---
        for r in range(4):
            wfc_t = []
            for i in range(NT):
                wfct = p_wfc.tile([P, 512], F32R, tag="wfc", name=f"wfc_{r}_{i}")
                nc.sync.dma_start(
                    out=wfct,
                    in_=wfcT[i * P:(i + 1) * P, r * 512:(r + 1) * 512])
                wfc_t.append(wfct)
            for t in range(NT):
                y_sb = p_y.tile([P, 512], F32, tag="y", name=f"ysb_{r}_{t}")
                yps = ps.tile([P, 512], F32, tag="ps", name=f"yps_{r}_{t}")
                for i in range(NT):
                    nc.tensor.matmul(
                        yps[:, :],
                        ot_t[i][:, t * P:(t + 1) * P],
                        wfc_t[i][:, r * 0:512],
                        start=(i == 0), stop=(i == NT - 1),
                    )
                nc.scalar.activation(y_sb[:, :], yps[:, :], AF.Copy)
                nc.sync.dma_start(out=y[t * P:(t + 1) * P, r * 512:(r + 1) * 512],
                                  in_=y_sb)


def _get_nc():
    if "nc" not in _CACHE:
        _CACHE["nc"] = _build()
    return _CACHE["nc"]


def kernel(q, k, v, Wq, Wk, Wv, Wfc, bfc):
    q = np.asarray(q, np.float32)
    k = np.asarray(k, np.float32)
    v = np.asarray(v, np.float32)
    Wq = np.asarray(Wq, np.float32)
    Wk = np.asarray(Wk, np.float32)
    Wv = np.asarray(Wv, np.float32)
    Wfc = np.asarray(Wfc, np.float32)
    bfc = np.asarray(bfc, np.float32)

    nc = _get_nc()
    qTb = [np.ascontiguousarray(q[b].T) for b in range(B)]
    kTb = [np.ascontiguousarray(k[b].T) for b in range(B)]
    vTb = [np.ascontiguousarray(v[b].T) for b in range(B)]
    wqTm = [np.ascontiguousarray((Wq[m * HO:(m + 1) * HO, :] / 8.0).T)
            for m in range(2)]
    wkTm = [np.ascontiguousarray(Wk[m * GO:(m + 1) * GO, :].T) for m in range(2)]
    wvTm = [np.ascontiguousarray(Wv[m * GO:(m + 1) * GO, :].T) for m in range(2)]
    wfcTm = [np.ascontiguousarray(Wfc[:, m * HO:(m + 1) * HO].T)
             for m in range(2)]

    in_maps = []
    for c in range(8):
        b, m = c // 2, c % 2
        in_maps.append({
            "qT": qTb[b], "kT": kTb[b], "vT": vTb[b],
            "wqT": wqTm[m], "wkT": wkTm[m], "wvT": wvTm[m],
            "wfcT": wfcTm[m],
        })
    res = run_bass_kernel_spmd(nc, in_maps, list(range(8)))
    out = np.empty((B, S, E), np.float32)
    for b in range(B):
        out[b] = res.results[2 * b]["y"] + res.results[2 * b + 1]["y"] + bfc
    return out

